# revision 13
# baseline (speedup 1.0000x reference)
"""Chebyshev approximation kernel for Trainium2 (8 NeuronCores, SPMD data-parallel).

Math: reference computes
    y_at_nodes = (1-t) * y[:, idx] + t * y[:, idx+1]      # [n_obs, deg]
    out        = (y_at_nodes @ basis).reshape(-1)         # [n_obs*deg]
Both steps are linear in y, so we fold them into a single matrix on host:
    C[k, d] = sum_j W[k, j] * basis[j, d],   W = interp weights (2 nnz/col)
    out     = y @ C          # [n_obs, 2049] @ [2049, 1024]
The device kernel is one GEMM per 128-row block: PE-transpose the y block
(grid axis onto partitions), then 17 accumulating float32r matmuls into PSUM.

Sharding: y rows split 8192/core across 8 cores; C replicated.
"""

import os
import numpy as np

DEG = 1024
N_OBS = 65536
M_P1 = 2049
N_CORES = 8
ROWS_PER_CORE = N_OBS // N_CORES  # 8192
KT = 17                           # contraction tiles of 128 (2049 -> 2176 padded)
KP = KT * 128                     # 2176
RB = 128                          # rows per block

_COMPILED = {}
LAST_RESULTS = None


def _cheb_c_matrix(x: np.ndarray) -> np.ndarray:
    """C [KP, DEG] float32 with zero pad rows >= 2049; out = y @ C[:M_P1]."""
    x = np.asarray(x, dtype=np.float32)
    k = np.arange(DEG, dtype=np.float32)
    # float32 node computation, mimicking the jax reference
    ang = (np.float32(np.pi) * (k + np.float32(0.5))) / np.float32(DEG)
    nodes = np.sort(np.cos(ang.astype(np.float32)).astype(np.float32))
    norm = ((np.float32(2.0) - (k == 0).astype(np.float32)) / np.float32(DEG)).astype(
        np.float64
    )
    # basis[j, d] = norm_d * cos(d * arccos(node_j)); f64 from f32 nodes
    theta = np.arccos(nodes.astype(np.float64))
    basis = norm[None, :] * np.cos(k.astype(np.float64)[None, :] * theta[:, None])
    idx = np.clip(np.searchsorted(x, nodes, side="right") - 1, 0, M_P1 - 2)
    a = x[idx]
    b = x[idx + 1]
    t = ((nodes - a) / (b - a)).astype(np.float64)
    C = np.zeros((KP, DEG), dtype=np.float64)
    np.add.at(C, idx, (1.0 - t)[:, None] * basis)
    np.add.at(C, idx + 1, t[:, None] * basis)
    return np.ascontiguousarray(C.astype(np.float32))


def build_cheb_kernel(tc, y_ap, c_ap, id_ap, o_ap, rows):
    """Emit the per-core program: out[rows, DEG] = y[rows, M_P1] @ C[:M_P1]."""
    import concourse.mybir as mybir

    nc = tc.nc
    f32 = mybir.dt.float32
    f32r = mybir.dt.float32r
    nblocks = rows // RB

    # 16 full k-tiles cover columns 0..2047; column 2048's rank-1 update is
    # folded on DVE during the output copy (out += y[:,2048] * C[2048,:]).
    KTM = 16
    # Stages grouped 4-per-PSUM-bank: a burst of 4 PE transposes shares one
    # PSUM bank, drained by a single wide copy; main matmuls run one group
    # behind so the drain is off their critical path.
    G = 4

    with (
        tc.tile_pool(name="consts", bufs=1) as consts,
        tc.tile_pool(name="ypool", bufs=3) as ypool,
        tc.tile_pool(name="ytpool", bufs=2) as ytpool,
        tc.tile_pool(name="opool", bufs=3) as opool,
        tc.tile_pool(name="pst", bufs=4, space="PSUM") as pstp,
        tc.tile_pool(name="pso", bufs=2, space="PSUM") as psop,
    ):
        ident = consts.tile([128, 128], f32r)
        nc.sync.dma_start(out=ident, in_=id_ap)
        # HAM warm-up: ~20 throwaway normal-mode matmuls on ident during the
        # DMA preamble so the PE clock is at 2.4 GHz when real work starts
        # (transpose-mode doesn't engage HAM).
        warm = psop.tile([128, DEG], f32, name="warm", tag="ps")
        for _ in range(20):
            nc.tensor.matmul(warm[:, 0:128], ident, ident, start=True, stop=True)
        # C resident in SBUF: [partition-within-tile, ktile, d]; chunked DMAs
        # on the scalar HWDGE queue so y loads (sync queue) aren't blocked.
        c_sb = consts.tile([128, KTM, DEG], f32r)
        c_r = c_ap.rearrange("(t p) n -> p t n", p=128)
        for k in range(KTM):
            nc.scalar.dma_start(out=c_sb[:, k, :], in_=c_r[:, k, :])
        # C row 2048 replicated across partitions for the DVE rank-1 fold.
        c_rep = consts.tile([128, DEG], f32)
        import concourse.bass as bass

        c_row = c_ap[KTM * 128 : KTM * 128 + 1, :].bitcast(f32)
        c_row_bc = bass.AP(
            tensor=c_row.tensor, offset=c_row.offset, ap=[[0, 128]] + list(c_row.ap[1:])
        )
        nc.scalar.dma_start(out=c_rep, in_=c_row_bc)

        ybs, ytbs, pss = {}, {}, {}

        def load_y(b, split=False):
            yb = ypool.tile([128, M_P1], f32r, name="yb", tag="yb")
            rows = y_ap[b * RB : (b + 1) * RB, :]
            if split:
                # halves so block 0's first transposes start sooner
                nc.sync.dma_start(out=yb[:, 0:1024], in_=rows[:, 0:1024])
                nc.sync.dma_start(out=yb[:, 1024:M_P1], in_=rows[:, 1024:M_P1])
            else:
                nc.sync.dma_start(out=yb, in_=rows)
            ybs[b] = yb

        def emit_t_group(b, g):
            if g == 0:
                ytbs[b] = ytpool.tile([128, KTM, 128], f32r, name="ytb", tag="ytb")
            pst = pstp.tile([128, G, 128], f32r, name="pst", tag="pst")
            for j in range(G):
                k = g * G + j
                nc.tensor.transpose(
                    pst[:, j, :], ybs[b][:, k * 128 : (k + 1) * 128], ident
                )
            dst = ytbs[b][:, g * G : (g + 1) * G, :]
            if g % 2 == 0:
                nc.vector.tensor_copy(dst, pst)
            else:
                nc.scalar.copy(dst, pst)

        def emit_m_group(b, g):
            if g == 0:
                pss[b] = psop.tile([128, DEG], f32, name="ps", tag="ps")
            ps = pss[b]
            for j in range(G):
                k = g * G + j
                for nh in range(2):
                    nc.tensor.matmul(
                        ps[:, nh * 512 : (nh + 1) * 512],
                        ytbs[b][:, k, :],
                        c_sb[:, k, nh * 512 : (nh + 1) * 512],
                        start=(k == 0),
                        stop=(k == KTM - 1),
                    )
            if g == KTM // G - 1:
                tmp = opool.tile([128, DEG], f32, name="tmp", tag="tmp")
                nc.vector.tensor_scalar_mul(
                    tmp, c_rep, ybs[b][:, 2048:2049].bitcast(f32)
                )
                osb = opool.tile([128, DEG], f32, name="osb", tag="osb")
                nc.vector.tensor_add(osb, ps, tmp)
                nc.scalar.dma_start(out=o_ap[b * RB : (b + 1) * RB, :], in_=osb)
                del ybs[b], ytbs[b], pss[b]

        groups = [(b, g) for b in range(nblocks) for g in range(KTM // G)]
        load_y(0, split=True)
        for i in range(len(groups) + 1):
            if i < len(groups):
                b, g = groups[i]
                if g == 0 and b + 1 < nblocks:
                    load_y(b + 1)
                emit_t_group(b, g)
            if i >= 1:
                emit_m_group(*groups[i - 1])


def _build_nc(rows):
    import concourse.mybir as mybir
    import concourse.tile as tile
    from concourse import bacc

    f32 = mybir.dt.float32
    f32r = mybir.dt.float32r
    nc = bacc.Bacc(
        "TRN2",
        target_bir_lowering=False,
        debug=False,
        enable_asserts=False,
        num_devices=N_CORES,
    )
    y_ap = nc.dram_tensor("y", [rows, M_P1], f32r, kind="ExternalInput").ap()
    c_ap = nc.dram_tensor("c", [KP, DEG], f32r, kind="ExternalInput").ap()
    id_ap = nc.dram_tensor("ident", [128, 128], f32r, kind="ExternalInput").ap()
    o_ap = nc.dram_tensor("o", [rows, DEG], f32, kind="ExternalOutput").ap()
    with tile.TileContext(nc) as tc:
        build_cheb_kernel(tc, y_ap, c_ap, id_ap, o_ap, rows)
    nc.compile()
    return nc


def _get_compiled(rows=ROWS_PER_CORE):
    if rows not in _COMPILED:
        _COMPILED[rows] = _build_nc(rows)
    return _COMPILED[rows]


def kernel(x: np.ndarray, y: np.ndarray) -> np.ndarray:
    global LAST_RESULTS
    from concourse import bass_utils

    x = np.asarray(x, dtype=np.float32)
    y = np.ascontiguousarray(np.asarray(y, dtype=np.float32))
    assert y.shape == (N_OBS, M_P1), y.shape
    C = _cheb_c_matrix(x)

    nc = _get_compiled()
    ident = np.ascontiguousarray(np.eye(128, dtype=np.float32))
    in_maps = [
        {"y": y[i * ROWS_PER_CORE : (i + 1) * ROWS_PER_CORE], "c": C, "ident": ident}
        for i in range(N_CORES)
    ]
    trace = bool(int(os.environ.get("CHEB_TRACE", "0")))
    res = bass_utils.run_bass_kernel_spmd(
        nc, in_maps, core_ids=list(range(N_CORES)), trace=trace
    )
    LAST_RESULTS = res
    out = np.concatenate([res.results[i]["o"] for i in range(N_CORES)], axis=0)
    return out.reshape(-1)
